# revision 11
# baseline (speedup 1.0000x reference)
"""Trainium2 Bass kernel for ConvexDisplacementUpdate (B=4, L=4096, D=256).

new_coords = alpha * softmax(10 * qhat @ khat^T) @ coords + (1-alpha) * coords
q = l2norm(latents @ Wq^T), k = l2norm(latents @ Wk^T)  (row-wise l2norm)

v3 strategy (vs the v2 kernel measured ~125us/iter on today's hardware):
  - same fp8 DoubleRow projections + scores (K=256 contraction, 0.5 cyc/row).
  - exp split by whole 1024-col slots: `act_slots` of the 64 (m-tile, l-half)
    slots run exact exp on ACT (bf16 out); the rest run the exp2 bit-trick as
    a SINGLE DVE tensor_scalar emitting int16 bf16-bitpatterns (i16 = s*
    inv_k*log2e*128 + (127-C)*128). This kills v2's separate bitcast-convert
    passes (~34us of Pool/DVE work per iteration).
  - norm chains rebalanced: q/k squares + sums on Pool (which can't touch
    PSUM but has slack), ALL ACT sqrts batched into one chunk so the
    activation table swaps exactly twice per iteration (Sqrt set <-> Exp set).
  - cross-iteration software pipelining: all phase-2-read tensors (lat8, k8,
    qh8, inv_k, ktrick, caug, weights) are double-buffered; phase 1 of
    iteration i+1 is emitted interleaved into phase 2 of iteration i at fixed
    slots, so the steady-state loop keeps ACT/DVE (the bottleneck engines)
    busy across the iteration boundary. The For_i body holds 2 iterations
    (A/B buffer ping-pong).
  - PSUM budget (8 banks): scores 2x[128,1024] (4) + pv [128,512] (1) +
    proj [128,1024] (2) + ssqk (1).
  - final alpha-blend + division on host (B*L*2 elements, trivial).

fp8-DoubleRow PV was tried and is ISA-illegal with tile_position quadrant
packing (s3_lw_dual_fp8_restrictions: dual-fp8 LdWeights needs >=16-col
stationary and no quadrant packing), and without packing it blows the PSUM
budget; PV therefore stays bf16.
"""

import numpy as np

B, L, D = 4, 4096, 256
HALF = L // 2  # 2048 query rows per core
NCORES = 8
INV_TEMP = 10.0
NLT = L // 128  # 32 m-tiles

_CACHE = {}


def build_module(loop_n=0, act_slots=32, hook_start=2, hook_spacing=2,
                 pv_lag=2, single=False, unroll=0, no_pv=False, body_iters=2):
    """Build + compile the SPMD Bass module (one program, 8 cores).

    act_slots: how many of the 64 (m-tile, l-half) exp slots run on ACT
    (exact exp); the rest use the DVE int16 exp2 bit-trick. Both write bf16
    into the same per-m-tile P tile.
    loop_n: if >0, wrap the body (TWO iterations, A/B buffers) in a hardware
    For_i loop for steady-state benchmarking.
    """
    import concourse.bacc as bacc
    import concourse.bass_isa as bass_isa
    import concourse.mybir as mybir
    import concourse.tile as tile
    from concourse.bass import ts
    from concourse.alu_op_type import AluOpType

    dt = mybir.dt
    f32 = dt.float32
    bf16 = dt.bfloat16
    fp8 = dt.float8e4
    i16 = dt.int16
    AF = mybir.ActivationFunctionType
    PM = mybir.MatmulPerfMode

    # exp2 bit-trick constants (bf16 bit pattern in int16):
    # exp(x) ~= bitcast_bf16(round(128*(x*log2e + 127 - C))), C centers the
    # piecewise-linear mantissa error (~3% max, softmax cancels most).
    LOG2E = 1.4426950408889634
    TRICK_SCALE = LOG2E * 128.0
    TRICK_BIAS = (127.0 - 0.0430) * 128.0

    # slot s (0..63) = (m-tile t = s//2, l-half h = s%2). ACT slots spread
    # evenly among the 64.
    act_set = set()
    acc = 0
    for s in range(64):
        acc += act_slots
        if acc >= 64:
            acc -= 64
            act_set.add(s)

    nc = bacc.Bacc("TRN2", target_bir_lowering=False, debug=False,
                   num_devices=NCORES)

    lat_d = nc.dram_tensor("lat8", [128, 2, L], fp8, kind="ExternalInput")
    wq_d = nc.dram_tensor("wq8", [128, 2, D], fp8, kind="ExternalInput")
    wk_d = nc.dram_tensor("wk8", [128, 2, D], fp8, kind="ExternalInput")
    caug_d = nc.dram_tensor("caug", [128, 3 * NLT], bf16, kind="ExternalInput")
    pv_d = nc.dram_tensor("pv", [3, HALF], f32, kind="ExternalOutput")

    with tile.TileContext(nc) as tc:
        from contextlib import ExitStack
        with ExitStack() as top:
            persist = top.enter_context(tc.tile_pool(name="persist", bufs=1))
            ssqk_ps = top.enter_context(
                tc.tile_pool(name="ssqk_ps", bufs=1, space="PSUM"))
            sp_ps = top.enter_context(
                tc.tile_pool(name="sp_ps", bufs=3, space="PSUM"))
            proj_ps = sp_ps
            pv_ps = top.enter_context(
                tc.tile_pool(name="pv_ps", bufs=1, space="PSUM"))
            sqk_pool = top.enter_context(tc.tile_pool(name="sqk", bufs=4))
            p_pool = top.enter_context(tc.tile_pool(name="p_sb", bufs=4))
            out_sb = top.enter_context(tc.tile_pool(name="out_sb", bufs=4))

            ones = persist.tile([128, 1], bf16, tag="ones", name="ones")
            nc.vector.memset(ones, 1.0)

            # double-buffered cross-phase tensors
            def mkset(bi):
                return {
                    "lat8": persist.tile([128, 2, L], fp8, tag=f"lat8_{bi}", name=f"lat8_{bi}"),
                    "w8q": persist.tile([128, 2, D], fp8, tag=f"w8q_{bi}", name=f"w8q_{bi}"),
                    "w8k": persist.tile([128, 2, D], fp8, tag=f"w8k_{bi}", name=f"w8k_{bi}"),
                    "caug": persist.tile([128, 3 * NLT], bf16, tag=f"caug_{bi}", name=f"caug_{bi}"),
                    "k8": persist.tile([128, 2, L], fp8, tag=f"k8_{bi}", name=f"k8_{bi}"),
                    "qh8": persist.tile([128, 2, HALF], fp8, tag=f"qh8_{bi}", name=f"qh8_{bi}"),
                    "ktrick": persist.tile([128, NLT], f32, tag=f"ktr_{bi}", name=f"ktr_{bi}"),
                    "qraw": [persist.tile([128, HALF], f32, tag=f"qraw{i}_{bi}", name=f"qraw{i}_{bi}")
                             for i in range(2)],
                    "sqq": [persist.tile([128, HALF], f32, tag=f"sqq{i}_{bi}", name=f"sqq{i}_{bi}")
                            for i in range(2)],
                    "sqsum": persist.tile([128, HALF + NLT], f32, tag=f"sqsum_{bi}", name=f"sqsum_{bi}"),
                    "nrm": persist.tile([128, HALF + NLT], f32, tag=f"nrm_{bi}", name=f"nrm_{bi}"),
                    "inv": persist.tile([128, HALF + NLT], f32, tag=f"inv_{bi}", name=f"inv_{bi}"),
                }
            sets = [mkset(0), mkset(1)]

            def phase1_chunks(bi):
                """List of closures; called in order (possibly interleaved
                into the other buffer's phase 2)."""
                P = sets[bi]
                chunks = []

                def dma_chunk():
                    nc.scalar.dma_start(out=P["w8q"], in_=wq_d[:, :, :])
                    nc.scalar.dma_start(out=P["w8k"], in_=wk_d[:, :, :])
                    nc.scalar.dma_start(out=P["caug"], in_=caug_d[:, :])
                    for off in range(0, L, 1024):
                        nc.sync.dma_start(out=P["lat8"][:, :, off:off + 1024],
                                          in_=lat_d[:, :, off:off + 1024])
                chunks.append(dma_chunk)

                def qproj_chunk(et, qb):
                    def go():
                        qp = proj_ps.tile([128, 1024], f32, tag="sp",
                                          name=f"qp{bi}_{et}_{qb}")
                        for hh in range(2):
                            sl = ts(2 * qb + hh, 512)
                            nc.tensor.matmul(qp[:, ts(hh, 512)],
                                             P["w8q"][:, :, ts(et, 128)],
                                             P["lat8"][:, :, sl],
                                             start=True, stop=True,
                                             perf_mode=PM.DoubleRow)
                        nc.vector.tensor_copy(
                            out=P["qraw"][et][:, ts(qb, 1024)], in_=qp)
                        nc.gpsimd.tensor_mul(P["sqq"][et][:, ts(qb, 1024)],
                                             P["qraw"][et][:, ts(qb, 1024)],
                                             P["qraw"][et][:, ts(qb, 1024)])
                    return go

                def qadd_chunk(cc):
                    def go():
                        sl = ts(cc, 1024)
                        nc.gpsimd.tensor_add(P["sqsum"][:, sl],
                                             P["sqq"][0][:, sl],
                                             P["sqq"][1][:, sl])
                    return go

                def qar_chunk(cc):
                    def go():
                        sl = ts(cc, 1024)
                        nc.gpsimd.partition_all_reduce(
                            P["sqsum"][:, sl], P["sqsum"][:, sl], channels=128,
                            reduce_op=bass_isa.ReduceOp.add)
                    return go
                for qb in range(2):
                    chunks.append(qproj_chunk(0, qb))
                    chunks.append(qproj_chunk(1, qb))
                    chunks.append(qadd_chunk(qb))

                ksq_tiles = {}

                def kproj_chunk(mbb, et):
                    def go():
                        kp = proj_ps.tile([128, 1024], f32, tag="sp",
                                          name=f"kp{bi}_{mbb}_{et}")
                        for hh in range(2):
                            nc.tensor.matmul(kp[:, ts(hh, 512)],
                                             P["w8k"][:, :, ts(et, 128)],
                                             P["lat8"][:, :, ts(2 * mbb + hh, 512)],
                                             start=True, stop=True,
                                             perf_mode=PM.DoubleRow)
                        nc.vector.tensor_copy(
                            out=P["k8"][:, et, ts(mbb, 1024)], in_=kp)
                        sq = sqk_pool.tile([128, 1024], bf16, tag="ksq",
                                           name=f"ksq{bi}_{mbb}_{et}")
                        nc.scalar.activation(sq, kp, AF.Square)
                        ksq_tiles[(mbb, et)] = sq
                    return go

                ssqk = ssqk_ps.tile([128, NLT], f32, tag="ssqk",
                                    name=f"ssqk{bi}")

                def churn_chunk(mbb):
                    def go():
                        sq0 = ksq_tiles.pop((mbb, 0))
                        sq1 = ksq_tiles.pop((mbb, 1))
                        for j in range(8):
                            col = 8 * mbb + j
                            nc.tensor.matmul(ssqk[:, col:col + 1],
                                             sq0[:, ts(j, 128)], ones,
                                             start=True, stop=False)
                            nc.tensor.matmul(ssqk[:, col:col + 1],
                                             sq1[:, ts(j, 128)], ones,
                                             start=False, stop=True)
                    return go
                for mbb in range(4):
                    chunks.append(kproj_chunk(mbb, 0))
                    chunks.append(kproj_chunk(mbb, 1))
                    chunks.append(churn_chunk(mbb))
                chunks.append(qar_chunk(0))
                chunks.append(qar_chunk(1))

                def norm_chunk():
                    # stage 0.01*ssqk into sqsum tail, then ONE fused Sqrt
                    # over [q-norms | k-norms] -> exactly 2 table
                    # swaps/iteration (the single instr can't be split), then
                    # ONE reciprocal. inv = [inv_q (HALF) | inv_k (NLT)].
                    nc.vector.tensor_scalar_mul(
                        P["sqsum"][:, HALF:HALF + NLT], ssqk,
                        1.0 / (INV_TEMP * INV_TEMP))
                    nc.scalar.activation(P["nrm"], P["sqsum"], AF.Sqrt)
                    nc.vector.reciprocal(P["inv"], P["nrm"])
                    nc.vector.tensor_scalar_mul(
                        P["ktrick"], P["inv"][:, HALF:HALF + NLT], TRICK_SCALE)
                chunks.append(norm_chunk)

                def qh8_chunk(et, qb):
                    def go():
                        sl = ts(qb, 1024)
                        nc.gpsimd.tensor_mul(P["qh8"][:, et, sl],
                                             P["qraw"][et][:, sl],
                                             P["inv"][:, sl])
                    return go
                for et in range(2):
                    for qb in range(2):
                        chunks.append(qh8_chunk(et, qb))
                return chunks

            def phase2(bi, hooks):
                """Scores -> exp -> PV for buffer bi; `hooks` maps slot ->
                list of phase-1 closures (next iteration) to emit there."""
                P = sets[bi]
                pv_all = pv_ps.tile([128, 512], f32, tag="pv", name=f"pv{bi}")
                sps = {}
                ptiles = {}

                def emit_sc(s):
                    t, h = divmod(s, 2)
                    sp = sp_ps.tile([128, 1024], f32, tag="sp",
                                    name=f"sp{bi}_{s}")
                    for hh in range(2):
                        lsl = slice(1024 * h + 512 * hh,
                                    1024 * h + 512 * hh + 512)
                        nc.tensor.matmul(sp[:, ts(hh, 512)],
                                         P["k8"][:, :, ts(t, 128)],
                                         P["qh8"][:, :, lsl],
                                         start=True, stop=True,
                                         perf_mode=PM.DoubleRow)
                    sps[s] = sp

                def emit_pv(t):
                    p = ptiles.pop(t)
                    if no_pv:
                        if t == NLT - 1:  # keep output defined: copy junk
                            nc.vector.tensor_copy(out=pv_all[0:3, :],
                                                  in_=p[0:3, 0:512])
                        return
                    for lb in range(4):
                        nc.tensor.matmul(pv_all[32 * lb:32 * lb + 3, :],
                                         P["caug"][:, 3 * t:3 * t + 3],
                                         p[:, ts(lb, 512)],
                                         start=(t == 0), stop=(t == NLT - 1),
                                         tile_position=(0, 32 * lb))

                emit_sc(0)
                emit_sc(1)
                for s in range(64):
                    t, h = divmod(s, 2)
                    if h == 0:
                        ptiles[t] = p_pool.tile([128, 2048], bf16, tag="p",
                                                name=f"p{bi}_{t}")
                    p = ptiles[t]
                    sp = sps.pop(s)
                    lo = 1024 * h
                    if s in act_set:
                        nc.scalar.activation(
                            p[:, lo:lo + 1024], sp, AF.Exp,
                            scale=P["inv"][:, HALF + t:HALF + t + 1])
                    else:
                        nc.vector.tensor_scalar(
                            out=p[:, lo:lo + 1024].bitcast(i16), in0=sp,
                            scalar1=P["ktrick"][:, t:t + 1],
                            scalar2=TRICK_BIAS,
                            op0=AluOpType.mult, op1=AluOpType.add)
                    if s + 2 < 64:
                        emit_sc(s + 2)
                    for ch in hooks.get(s, ()):
                        ch()
                    if h == 1 and t >= pv_lag:
                        emit_pv(t - pv_lag)
                for t in range(NLT - pv_lag, NLT):
                    emit_pv(t)

                for lb in range(4):
                    ot = out_sb.tile([3, 512], f32, tag="ot", name=f"ot{bi}_{lb}")
                    if lb % 2 == 0:
                        nc.vector.tensor_copy(
                            out=ot, in_=pv_all[32 * lb:32 * lb + 3, :])
                        nc.sync.dma_start(out=pv_d[:, ts(lb, 512)], in_=ot)
                    else:
                        nc.scalar.activation(
                            ot, pv_all[32 * lb:32 * lb + 3, :], AF.Copy)
                        nc.scalar.dma_start(out=pv_d[:, ts(lb, 512)], in_=ot)

            def schedule(chunks):
                sched = {}
                s = hook_start
                for ch in chunks:
                    sched.setdefault(min(s, 63), []).append(ch)
                    s += hook_spacing
                return sched

            # preamble: phase 1 of buffer 0, emitted sequentially
            for ch in phase1_chunks(0):
                ch()

            if loop_n:
                assert body_iters % 2 == 0
                with tc.For_i(0, loop_n, 1):
                    for _bi in range(body_iters // 2):
                        phase2(0, schedule(phase1_chunks(1)))
                        phase2(1, schedule(phase1_chunks(0)))
            elif unroll:
                for _r in range(unroll):
                    phase2(0, schedule(phase1_chunks(1)))
                    phase2(1, schedule(phase1_chunks(0)))
            elif single:
                phase2(0, {})
            else:
                phase2(0, schedule(phase1_chunks(1)))
                phase2(1, {})

    nc.compile()
    return nc


def _get_module():
    if "nc" not in _CACHE:
        _CACHE["nc"] = build_module(single=True)
    return _CACHE["nc"]


def make_in_maps(latents, current_coords, Wq, Wk):
    """Per-core input dicts. Core c -> batch c//2, query half c%2 (rolled
    so own query rows are always columns 0:2048)."""
    import ml_dtypes
    fp8 = ml_dtypes.float8_e4m3fn
    latents = np.asarray(latents, np.float32)
    coords = np.asarray(current_coords, np.float32)

    def dhalves(mat_T):  # [256, N] -> [128, 2, N] (partition, d-half, col)
        return np.ascontiguousarray(
            mat_T.reshape(2, 128, -1).transpose(1, 0, 2)).astype(fp8)

    wq8 = dhalves(np.ascontiguousarray(np.asarray(Wq, np.float32).T))
    wk8 = dhalves(np.ascontiguousarray(np.asarray(Wk, np.float32).T))
    in_maps = []
    for c in range(NCORES):
        b, h = divmod(c, 2)
        lat_b = np.roll(latents[b], -HALF * h, axis=0)
        coo_b = np.roll(coords[b], -HALF * h, axis=0)
        aug = np.concatenate([coo_b, np.ones((L, 1), np.float32)], axis=1)
        caug = np.ascontiguousarray(
            aug.reshape(L // 128, 128, 3).transpose(1, 0, 2).reshape(128, -1))
        in_maps.append({
            "lat8": dhalves(np.ascontiguousarray(lat_b.T)),
            "wq8": wq8,
            "wk8": wk8,
            "caug": caug.astype(ml_dtypes.bfloat16),
        })
    return in_maps


def postprocess(results, current_coords, alpha):
    """Assemble (new_coords, displacement) from per-core pv = [num_x; num_y; den]."""
    coords = np.asarray(current_coords, np.float32)
    new_coords = np.empty((B, L, 2), np.float32)
    for c in range(NCORES):
        b, h = divmod(c, 2)
        pv = results[c]["pv"]
        wc = (pv[0:2, :] / pv[2:3, :]).T  # [2048, 2] = (W @ coords) rows
        rows = slice(h * HALF, (h + 1) * HALF)
        new_coords[b, rows] = alpha * wc + (1.0 - alpha) * coords[b, rows]
    displacement = new_coords - coords
    return new_coords, displacement


def kernel(latents, current_coords, Wq, Wk, alpha_raw, layer_idx=None):
    from concourse.bass_utils import run_bass_kernel_spmd

    nc = _get_module()
    in_maps = make_in_maps(latents, current_coords, Wq, Wk)
    res = run_bass_kernel_spmd(nc, in_maps, list(range(NCORES)))
    alpha = np.float32(1.0 / (1.0 + np.exp(-np.float64(np.asarray(alpha_raw)))))
    return postprocess(res.results, current_coords, alpha)
